# revision 60
# baseline (speedup 1.0000x reference)
"""ABCNN forward kernel for 8 Trainium2 NeuronCores (data-parallel over batch).

Hardcoded problem: B=256, L=256, D=100, V=50000, NUM_LAYER=2, LIN=512.
Each core processes 32 batch items end-to-end; no collectives.

Restructured vs the first version:
- d2 rank-1 terms folded into the cross matmul via 2 extra contraction rows
  (sp rows 100/101 hold [-0.5*sq1 | 1] and [1 | -0.5*sq2]); psum = -0.5*d2.
- A^T produced by swapped-operand matmuls + a second activation chain
  (no PE transposes of A).
- Conv factored as conv_f = sum_dy shift_dy(A @ W) @ M_dy: F = A@W computed
  once (2 matmuls, W stationary), then 3 banded matmuls + 3 e-channel banded
  matmuls, all N=512.
- Chain shortened: dmax = max(-2*psum, 0); r = ARS(dmax+eps); s = dmax*r;
  t = 2s + dmax; A = ARS(t + 1) = 1/(1+sqrt(dmax)).  Only ARS-set scalar
  functions; tanh grouped in per-16-batch waves to avoid table thrashing.
- Last layer skips match2/pooling/e-update entirely (dead code in reference).
- Host precomputes: layer-0 masked sp (incl. sq rows), v-broadcasts, initial
  e-means (res pieces 0/3), padded embeddings.
"""
import sys
import numpy as np

sys.path.insert(0, "/opt/trn_rl_repo")

B, L, D, V = 256, 256, 100, 50000
NL = 2
LIN = 512
EPS = 1e-5
NCORES = 8
PB = B // NCORES   # 32 batches per core
DEPS = 1e-12       # eps inside first abs_rsqrt of the chain
GRP = 16           # batches per scalar-table wave
WPAD = 260         # padded row width for shifted reads: [2 zeros, 256, 2 zeros]

_CACHE = {}


def _bf16():
    import ml_dtypes
    return ml_dtypes.bfloat16


def _build(cb_vals):
    """Build the Bass/Tile graph once. Returns nc."""
    import concourse.bass as bass
    import concourse.mybir as mybir
    from concourse import bacc, tile

    dt = mybir.dt
    AF = mybir.ActivationFunctionType
    OP = mybir.AluOpType
    AX = mybir.AxisListType
    ARS = AF.Abs_reciprocal_sqrt

    nc = bacc.Bacc("TRN2")

    # ---------------- DRAM inputs (per-core shard shapes) ------------------
    d_es = nc.dram_tensor("es", [PB, D, 2 * WPAD], dt.bfloat16, kind="ExternalInput")
    d_vrs = nc.dram_tensor("vrs", [PB, D, 2 * L], dt.bfloat16, kind="ExternalInput")
    d_sp0 = nc.dram_tensor("sp0", [PB, D, 2 * L], dt.bfloat16, kind="ExternalInput")
    d_sq0 = nc.dram_tensor("sq0", [PB, 1, 2 * L], dt.bfloat16, kind="ExternalInput")
    d_res03 = nc.dram_tensor("res03", [2, D, PB], dt.float32, kind="ExternalInput")
    d_wch = nc.dram_tensor("wch", [NL, 2, 128, D], dt.bfloat16, kind="ExternalInput")
    d_mdy = nc.dram_tensor("mdy", [NL, 3, D, D], dt.bfloat16, kind="ExternalInput")
    d_edy = nc.dram_tensor("edy", [NL, 3, D, D], dt.bfloat16, kind="ExternalInput")
    d_fc1w = nc.dram_tensor("fc1w", [6, D, LIN], dt.bfloat16, kind="ExternalInput")
    d_fc1b = nc.dram_tensor("fc1b", [PB, LIN], dt.bfloat16, kind="ExternalInput")
    d_lng = nc.dram_tensor("lng", [PB, LIN], dt.bfloat16, kind="ExternalInput")
    d_lnb = nc.dram_tensor("lnb", [PB, LIN], dt.bfloat16, kind="ExternalInput")
    d_fc2w = nc.dram_tensor("fc2w", [4, 128, 2], dt.bfloat16, kind="ExternalInput")
    d_fc2b = nc.dram_tensor("fc2b", [2, 1], dt.float32, kind="ExternalInput")
    d_ident32 = nc.dram_tensor("ident32", [128, 128], dt.float32, kind="ExternalInput")
    d_out = nc.dram_tensor("out", [2, PB], dt.float32, kind="ExternalOutput")

    with tile.TileContext(nc) as tc:
        with (
            tc.tile_pool(name="pers", bufs=1) as pers,
            tc.tile_pool(name="aa", bufs=GRP + 2) as aapool,
            tc.tile_pool(name="tr", bufs=2) as tr,
            tc.tile_pool(name="hd", bufs=1) as hd,
            tc.tile_pool(name="ps_a", bufs=4, space="PSUM") as ps_a,
            tc.tile_pool(name="ps_fc", bufs=2, space="PSUM") as ps_fc,
            tc.tile_pool(name="ps_sml", bufs=2, space="PSUM") as ps_sml,
        ):
            # ---------------- persistent tiles -----------------------------
            es, os_, vrs = [], [], []
            for b in range(PB):
                es.append(pers.tile([D, 2 * WPAD], dt.bfloat16, tag=f"e_{b}", name=f"e_{b}"))
                os_.append(pers.tile([D, 2 * L], dt.bfloat16, tag=f"o_{b}", name=f"o_{b}"))
                vrs.append(pers.tile([D, 2 * L], dt.bfloat16, tag=f"vr_{b}", name=f"vr_{b}"))
            wch = pers.tile([128, NL * 2 * D], dt.bfloat16, tag="wch", name="wch")
            mdy = pers.tile([D, NL * 3 * D], dt.bfloat16, tag="mdy", name="mdy")
            edy = pers.tile([D, NL * 3 * D], dt.bfloat16, tag="edy", name="edy")
            fc1w = pers.tile([D, 6 * LIN], dt.bfloat16, tag="fc1w", name="fc1w")
            fc1b = pers.tile([PB, LIN], dt.bfloat16, tag="fc1b", name="fc1b")
            lng = pers.tile([PB, LIN], dt.bfloat16, tag="lng", name="lng")
            lnb = pers.tile([PB, LIN], dt.bfloat16, tag="lnb", name="lnb")
            fc2w = pers.tile([128, 4 * 2], dt.bfloat16, tag="fc2w", name="fc2w")
            fc2b = pers.tile([2, 1], dt.float32, tag="fc2b", name="fc2b")
            ident32 = pers.tile([128, 128], dt.float32, tag="ident32", name="ident32")
            resX = [pers.tile([D, PB], dt.float32, tag=f"res_{p}", name=f"res_{p}")
                    for p in range(6)]
            # y and F staging buffers, double-buffered by batch parity,
            # with zeroed border columns for shifted reads.
            ybuf = [pers.tile([D, 2 * WPAD], dt.bfloat16, tag=f"y_{i}", name=f"y_{i}")
                    for i in range(2)]
            fsb = [pers.tile([D, 2 * WPAD], dt.bfloat16, tag=f"fs_{i}", name=f"fs_{i}")
                   for i in range(2)]
            ones_Dx1 = pers.tile([D, 1], dt.bfloat16, tag="c_onesd1", name="c_onesd1")
            ones_128x1 = pers.tile([128, 1], dt.bfloat16, tag="c_ones128", name="c_ones128")
            third_1xD = pers.tile([1, D], dt.bfloat16, tag="c_third", name="c_third")
            # ext-row operand pair for the rank-2 d2 term (K=33 matmul):
            # EL: row0 = -0.5*sq (per match), row32 = 1, rows 1..31 = 0
            # ER: row0 = 1, row32 = -0.5*sq (per match), rows 1..31 = 0
            ELs = [pers.tile([33, 2 * L], dt.bfloat16, tag=f"EL_{i}", name=f"EL_{i}")
                   for i in range(4)]
            ERs = [pers.tile([33, 2 * L], dt.bfloat16, tag=f"ER_{i}", name=f"ER_{i}")
                   for i in range(4)]
            c_deps = pers.tile([128, 1], dt.float32, tag="c_deps", name="c_deps")
            c_eps = pers.tile([128, 1], dt.float32, tag="c_eps", name="c_eps")
            c_cb = [pers.tile([128, 1], dt.float32, tag=f"c_cb{l}", name=f"c_cb{l}")
                    for l in range(NL)]

            nc.vector.memset(ones_Dx1[:], 1.0)
            nc.vector.memset(ones_128x1[:], 1.0)
            nc.vector.memset(third_1xD[:], 1.0 / 3.0)
            nc.vector.memset(c_deps[:], DEPS)
            nc.vector.memset(c_eps[:], EPS)
            for l in range(NL):
                nc.vector.memset(c_cb[l][:], float(cb_vals[l]))
            for i in range(2):
                nc.vector.memset(ybuf[i][:], 0.0)
                nc.vector.memset(fsb[i][:], 0.0)
            for i in range(4):
                nc.vector.memset(ELs[i][:], 0.0)
                nc.vector.memset(ERs[i][:], 0.0)
                nc.vector.memset(ELs[i][32:33, :], 1.0)
                nc.vector.memset(ERs[i][0:1, :], 1.0)

            # ---------------- load weights (es/vrs staggered into waves) ---
            for l in range(NL):
                for c in range(2):
                    nc.sync.dma_start(
                        wch[:, (l * 2 + c) * D:(l * 2 + c + 1) * D],
                        d_wch[l, c, :, :])
                for dy in range(3):
                    nc.sync.dma_start(
                        mdy[:, (l * 3 + dy) * D:(l * 3 + dy + 1) * D],
                        d_mdy[l, dy, :, :])
                    nc.sync.dma_start(
                        edy[:, (l * 3 + dy) * D:(l * 3 + dy + 1) * D],
                        d_edy[l, dy, :, :])
            for p in range(6):
                nc.sync.dma_start(fc1w[:, p * LIN:(p + 1) * LIN], d_fc1w[p, :, :])
            nc.sync.dma_start(fc1b[:], d_fc1b[:, :])
            nc.sync.dma_start(lng[:], d_lng[:, :])
            nc.sync.dma_start(lnb[:], d_lnb[:, :])
            for q in range(4):
                nc.sync.dma_start(fc2w[:, q * 2:(q + 1) * 2], d_fc2w[q, :, :])
            nc.sync.dma_start(fc2b[:], d_fc2b[:, :])
            nc.sync.dma_start(ident32[:], d_ident32[:, :])
            nc.sync.dma_start(resX[0][:], d_res03[0, :, :])
            nc.sync.dma_start(resX[3][:], d_res03[1, :, :])

            # ---------------- helpers --------------------------------------
            def chain(dp, out_ap, mid_on_gpsimd, clamp_on_scalar=False):
                """A = 1/(1+sqrt(max(-2*psum,0))) from a [128,512] PSUM tile;
                writes bf16 A into out_ap."""
                eng = nc.gpsimd if mid_on_gpsimd else nc.vector
                dmax = tr.tile([128, 2 * L], dt.bfloat16, tag="dmax", name="dmax", bufs=3)
                if clamp_on_scalar:
                    nc.scalar.activation(dmax[:], dp[:], AF.Relu, scale=-2.0)
                else:
                    nc.vector.tensor_scalar(dmax[:], dp[:], -2.0, 0.0, OP.mult, OP.max)
                # r = 2/sqrt(dmax) via ARS(0.25*x); s = dmax*r = 2*sqrt(dmax)
                r = tr.tile([128, 2 * L], dt.bfloat16, tag="chr", name="chr", bufs=3)
                nc.scalar.activation(r[:], dmax[:], ARS, bias=c_deps[:], scale=0.25)
                s = tr.tile([128, 2 * L], dt.bfloat16, tag="chs", name="chs", bufs=3)
                eng.tensor_tensor(s[:], dmax[:], r[:], OP.mult)
                t = tr.tile([128, 2 * L], dt.bfloat16, tag="cht", name="cht", bufs=3)
                eng.tensor_tensor(t[:], s[:], dmax[:], OP.add)
                nc.scalar.activation(out_ap, t[:], ARS, bias=1.0)

            def ext_fill(b, spq_src):
                """sq-sums of the match operands -> EL row0 / ER row32."""
                el, er = ELs[(b % 2) + 2 * (b // GRP)], ERs[(b % 2) + 2 * (b // GRP)]
                sqt = ps_sml.tile([32, 2 * L], dt.float32, tag="sml", name="sqt")
                nc.tensor.matmul(sqt[0:1, :], ones_Dx1[:], spq_src[:],
                                 start=True, stop=True)
                nc.vector.tensor_scalar_mul(el[0:1, :], sqt[0:1, :], -0.5)
                nc.vector.tensor_scalar_mul(er[32:33, :], sqt[0:1, :], -0.5)
                return el, er

            def d2_mms(dp, sp, el, er, transposed):
                """psum = -0.5 * d2 via cross (K=100) + ext (K=33) matmuls."""
                src, dst = (L, 0) if transposed else (0, L)
                for c in range(2):
                    sl = dp[:, c * L:(c + 1) * L]
                    nc.tensor.matmul(sl, sp[:, src + c * 128:src + (c + 1) * 128],
                                     sp[:, dst:dst + L], start=True, stop=False)
                    nc.tensor.matmul(sl, el[:, src + c * 128:src + (c + 1) * 128],
                                     er[:, dst:dst + L], start=False, stop=True)

            # ---------------- per-batch stage emitters ----------------------
            aas = {}

            def p1_batch(l, b, clamp_on_scalar=False):
                """Match on embeddings -> A and A^T (ARS-set scalar only)."""
                sp = tr.tile([D, 2 * L], dt.bfloat16, tag="sp",
                             name="sp", bufs=6)
                if l == 0:
                    nc.sync.dma_start(sp[:], d_sp0[b, :, :])
                    el = ELs[(b % 2) + 2 * (b // GRP)]
                    er = ERs[(b % 2) + 2 * (b // GRP)]
                    nc.sync.dma_start(el[0:1, :], d_sq0[b, :, :])
                    nc.sync.dma_start(er[32:33, :], d_sq0[b, :, :])
                else:
                    ev = es[b][:].rearrange("p (c w) -> p c w", c=2)
                    vv = vrs[b][:].rearrange("p (c w) -> p c w", c=2)
                    spv = sp[:].rearrange("p (c w) -> p c w", c=2)
                    nc.vector.tensor_tensor(spv, ev[:, :, 2:258], vv, OP.mult)
                    spq = tr.tile([D, 2 * L], dt.bfloat16, tag="spq",
                                  name="spq", bufs=2)
                    nc.gpsimd.tensor_tensor(spq[:], sp[:], sp[:], OP.mult)
                    el, er = ext_fill(b, spq)
                aa = aapool.tile([128, 4 * L], dt.bfloat16, tag="AA", name="AA")
                aas[(l, b)] = aa
                # A: psum[i_chunk, j] ; A^T: psum[j_chunk, i]
                dp = ps_a.tile([128, 2 * L], dt.float32, tag="d2", name="dp")
                d2_mms(dp, sp, el, er, transposed=False)
                chain(dp, aa[:, 2 * L:4 * L], mid_on_gpsimd=False,
                      clamp_on_scalar=clamp_on_scalar)
                dpT = ps_a.tile([128, 2 * L], dt.float32, tag="d2", name="dpT")
                d2_mms(dpT, sp, el, er, transposed=True)
                chain(dpT, aa[:, 0:2 * L], mid_on_gpsimd=True,
                      clamp_on_scalar=clamp_on_scalar)

            def conv_batch(l, b):
                """F = A@W, banded conv, tanh (tanh-set scalar only)."""
                if True:
                    if True:
                        aa = aas[(l, b)]
                        fpt = ps_fc.tile([128, 2 * L], dt.float32, tag="fcv", name="fpt")
                        fp = fpt[0:D, :]
                        fpv = fp.rearrange("p (c w) -> p c w", c=2)
                        aav = aa[:].rearrange("p (x y w) -> p x y w", x=2, y=2)
                        for c in range(2):
                            # rhs: [128, 2, 256] = {AT chunk c, A chunk c}
                            nc.tensor.matmul(
                                fpv, wch[:, (l * 2 + c) * D:(l * 2 + c + 1) * D],
                                aav[:, :, c, :], start=(c == 0), stop=(c == 1))
                        fs = fsb[b % 2]
                        fsv = fs[:].rearrange("p (c w) -> p c w", c=2)
                        nc.vector.tensor_copy(fsv[:, :, 2:258], fpv)
                        cvt = ps_fc.tile([128, 2 * L], dt.float32, tag="fcv", name="cvt")
                        cv = cvt[0:D, :]
                        cvv = cv.rearrange("p (c w) -> p c w", c=2)
                        ev = es[b][:].rearrange("p (c w) -> p c w", c=2)
                        for dy in range(3):
                            nc.tensor.matmul(
                                cvv, mdy[:, (l * 3 + dy) * D:(l * 3 + dy + 1) * D],
                                fsv[:, :, 1 + dy:257 + dy],
                                start=(dy == 0), stop=False)
                            nc.tensor.matmul(
                                cvv, edy[:, (l * 3 + dy) * D:(l * 3 + dy + 1) * D],
                                ev[:, :, 1 + dy:257 + dy],
                                start=False, stop=(dy == 2))
                        for (side, pieceidx) in ((0, 1 + l), (1, 4 + l)):
                            nc.scalar.activation(
                                os_[b][:, side * L:(side + 1) * L],
                                cv[:, side * L:(side + 1) * L], AF.Tanh,
                                bias=c_cb[l][0:D],
                                accum_out=resX[pieceidx][:, b:b + 1])

            p3state = {}

            def p3_front(b):
                """Match on conv outputs through the A2 chain (ARS-set only)."""
                if True:
                    for _ in range(1):
                        sp = tr.tile([D, 2 * L], dt.bfloat16, tag="sp",
                                     name="sp", bufs=6)
                        nc.vector.tensor_tensor(sp[:], os_[b][:], vrs[b][:],
                                                OP.mult)
                        spq = tr.tile([D, 2 * L], dt.bfloat16, tag="spq",
                                      name="spq", bufs=2)
                        nc.gpsimd.tensor_tensor(spq[:], sp[:], sp[:], OP.mult)
                        el, er = ext_fill(b, spq)
                        dp2 = ps_a.tile([128, 2 * L], dt.float32, tag="d2", name="dp2")
                        d2_mms(dp2, sp, el, er, transposed=False)
                        # chain with mids on gpsimd; final split w/ accum -> w1c
                        dmax = tr.tile([128, 2 * L], dt.bfloat16, tag="dmax",
                                       name="dmax", bufs=3)
                        nc.scalar.activation(dmax[:], dp2[:], AF.Relu, scale=-2.0)
                        r = tr.tile([128, 2 * L], dt.bfloat16, tag="chr",
                                    name="chr", bufs=3)
                        nc.scalar.activation(r[:], dmax[:], ARS, bias=c_deps[:],
                                             scale=0.25)
                        s = tr.tile([128, 2 * L], dt.bfloat16, tag="chs3",
                                    name="chs3", bufs=2)
                        nc.vector.tensor_tensor(s[:], dmax[:], r[:], OP.mult)
                        t = tr.tile([128, 2 * L], dt.bfloat16, tag="cht3",
                                    name="cht3", bufs=2)
                        nc.gpsimd.tensor_tensor(t[:], s[:], dmax[:], OP.add)
                        a2 = tr.tile([128, 2 * L], dt.bfloat16, tag="A2",
                                     name="a2", bufs=5)
                        w1c = tr.tile([128, 2], dt.float32, tag="w1c",
                                      name="w1c", bufs=5)
                        for c in range(2):
                            nc.scalar.activation(
                                a2[:, c * L:(c + 1) * L], t[:, c * L:(c + 1) * L],
                                ARS, bias=1.0, accum_out=w1c[:, c:c + 1])
                        p3state[b] = (a2, w1c)

            def p3_tail(b):
                """w1/w2 weighting, pooling, e-update for an earlier p3_front."""
                a2, w1c = p3state.pop(b)
                if True:
                    for _ in range(1):
                        # w2 = col sums of A2 (over i), via ones matmuls
                        w2t = ps_sml.tile([32, 2 * L], dt.float32, tag="sml",
                                          name="w2t")
                        nc.tensor.matmul(w2t[0:1, 0:L], ones_128x1[:], a2[:, 0:L],
                                         start=True, stop=False)
                        nc.tensor.matmul(w2t[0:1, 0:L], ones_128x1[:], a2[:, L:2 * L],
                                         start=False, stop=True)
                        w2row = tr.tile([1, L], dt.bfloat16, tag="w2row", name="w2row")
                        nc.vector.tensor_copy(w2row[:], w2t[0:1, 0:L])
                        # w1 rows via two fp32 column transposes
                        w1row = tr.tile([1, L], dt.bfloat16, tag="w1row", name="w1row")
                        for c in range(2):
                            w1t = ps_sml.tile([32, 2 * L], dt.float32, tag="sml",
                                              name="w1t")
                            nc.tensor.transpose(w1t[0:1, 0:128], w1c[:, c:c + 1],
                                                ident32[:])
                            nc.vector.tensor_copy(w1row[0:1, c * 128:(c + 1) * 128],
                                                  w1t[0:1, 0:128])
                        # wr = (1/3) * [w1 bcast | w2 bcast] over partitions
                        wr = ps_sml.tile([D, 2 * L], dt.float32, tag="sml", name="wr")
                        nc.tensor.matmul(wr[:, 0:L], third_1xD[:], w1row[:],
                                         start=True, stop=True)
                        nc.tensor.matmul(wr[:, L:2 * L], third_1xD[:], w2row[:],
                                         start=True, stop=True)
                        # y = o * wr (into padded buffer), pool, e += pooled
                        y = ybuf[b % 2]
                        yv = y[:].rearrange("p (c w) -> p c w", c=2)
                        ov = os_[b][:].rearrange("p (c w) -> p c w", c=2)
                        wv = wr[:].rearrange("p (c w) -> p c w", c=2)
                        nc.vector.tensor_tensor(yv[:, :, 2:258], ov, wv, OP.mult)
                        u = tr.tile([D, 2 * L], dt.bfloat16, tag="u", name="u", bufs=2)
                        uv = u[:].rearrange("p (c w) -> p c w", c=2)
                        nc.gpsimd.tensor_tensor(uv, yv[:, :, 1:257], yv[:, :, 3:259],
                                                OP.add)
                        u2 = tr.tile([D, 2 * L], dt.bfloat16, tag="u2", name="u2",
                                     bufs=2)
                        u2v = u2[:].rearrange("p (c w) -> p c w", c=2)
                        nc.gpsimd.tensor_tensor(u2v, uv, yv[:, :, 2:258], OP.add)
                        evv = es[b][:].rearrange("p (c w) -> p c w", c=2)[:, :, 2:258]
                        nc.vector.tensor_tensor(evv, evv, u2v, OP.add)

            # ---------------- wave sequence --------------------------------
            # Scalar activations are grouped by table set per wave (ARS vs
            # tanh); no_sync_barrier keeps the scheduler from interleaving
            # them, which would thrash the activation-table RAM.  P3 of one
            # group shares a wave with the NEXT group's P1 (both ARS-only)
            # so the PE always has independent matmuls queued.
            g0 = list(range(GRP))
            g1 = list(range(GRP, PB))
            tc.no_sync_barrier()
            for b in g0:
                p1_batch(0, b)
            tc.no_sync_barrier()
            # stagger the non-critical input DMAs behind the first wave
            for b in g0:
                nc.sync.dma_start(es[b][:], d_es[b, :, :])
            for b in g0:
                conv_batch(0, b)
            tc.no_sync_barrier()
            for b in g1:
                nc.sync.dma_start(es[b][:], d_es[b, :, :])
            for b in range(PB):
                nc.sync.dma_start(vrs[b][:], d_vrs[b, :, :])
            # P3 tails run one pair late so their matmuls never block the
            # in-order PE queue on unresolved chain dependencies.
            for i, b in enumerate(g0):
                p1_batch(0, g1[i], clamp_on_scalar=True)
                p3_front(b)
                if i > 2:
                    p3_tail(g0[i - 3])
            p3_tail(g0[-3])
            p3_tail(g0[-2])
            p3_tail(g0[-1])
            tc.no_sync_barrier()
            for b in g1:
                conv_batch(0, b)
            tc.no_sync_barrier()
            for i, b in enumerate(g1):
                p1_batch(1, g0[i], clamp_on_scalar=True)
                p3_front(b)
                if i > 2:
                    p3_tail(g1[i - 3])
            p3_tail(g1[-3])
            p3_tail(g1[-2])
            p3_tail(g1[-1])
            tc.no_sync_barrier()
            for b in g0:
                conv_batch(1, b)
            tc.no_sync_barrier()
            for b in g1:
                p1_batch(1, b, clamp_on_scalar=True)
            tc.no_sync_barrier()
            for b in g1:
                conv_batch(1, b)
            tc.no_sync_barrier()

            # ---------------- output head ---------------------------------
            hps = ps_sml.tile([32, 2 * L], dt.float32, tag="sml", name="hps")
            for p in range(6):
                rb = hd.tile([D, PB], dt.bfloat16, tag="resbf", name="resbf")
                nc.vector.tensor_scalar_mul(rb[:], resX[p][:], 1.0 / 256.0)
                nc.tensor.matmul(hps[:], rb[:], fc1w[:, p * LIN:(p + 1) * LIN],
                                 start=(p == 0), stop=(p == 5))
            hb = hd.tile([PB, LIN], dt.float32, tag="lnA", name="hb")
            nc.vector.tensor_tensor(hb[:], hps[:], fc1b[:], OP.add)
            musum = hd.tile([PB, 1], dt.float32, tag="musum", name="musum")
            nc.vector.tensor_reduce(musum[:], hb[:], AX.X, OP.add)
            mu = hd.tile([PB, 1], dt.float32, tag="mu", name="mu")
            nc.vector.tensor_scalar_mul(mu[:], musum[:], 1.0 / LIN)
            hc = hd.tile([PB, LIN], dt.float32, tag="lnB", name="hc")
            nc.vector.tensor_scalar(hc[:], hb[:], mu[:], None, OP.subtract)
            sq = hd.tile([PB, LIN], dt.float32, tag="lnA", name="sqh")
            nc.vector.tensor_tensor(sq[:], hc[:], hc[:], OP.mult)
            vsum = hd.tile([PB, 1], dt.float32, tag="vsum", name="vsum")
            nc.vector.tensor_reduce(vsum[:], sq[:], AX.X, OP.add)
            rstd = hd.tile([PB, 1], dt.float32, tag="rstd", name="rstd")
            nc.scalar.activation(rstd[:], vsum[:], ARS, bias=c_eps[0:PB], scale=1.0 / LIN)
            hn = hd.tile([PB, LIN], dt.float32, tag="lnA", name="hn")
            nc.vector.tensor_scalar(hn[:], hc[:], rstd[:], None, OP.mult)
            hg = hd.tile([PB, LIN], dt.float32, tag="lnB", name="hg")
            nc.vector.tensor_tensor(hg[:], hn[:], lng[:], OP.mult)
            hgb = hd.tile([PB, LIN], dt.float32, tag="lnA", name="hgb")
            nc.vector.tensor_tensor(hgb[:], hg[:], lnb[:], OP.add)
            hr = hd.tile([PB, LIN], dt.float32, tag="hr", name="hr")
            nc.vector.tensor_scalar_max(hr[:], hgb[:], 0.0)
            out2 = ps_sml.tile([32, 2 * L], dt.float32, tag="sml", name="out2")
            for q in range(4):
                htp = ps_fc.tile([128, 2 * L], dt.float32, tag="fcv", name="htp")
                nc.tensor.transpose(htp[:, 0:PB], hr[:, q * 128:(q + 1) * 128],
                                    ident32[0:PB, 0:PB])
                ht = hd.tile([128, PB], dt.bfloat16, tag="ht", name="ht")
                nc.vector.tensor_copy(ht[:], htp[:, 0:PB])
                nc.tensor.matmul(out2[0:2, 0:PB], fc2w[:, q * 2:(q + 1) * 2], ht[:],
                                 start=(q == 0), stop=(q == 3))
            outsb = hd.tile([2, PB], dt.float32, tag="outsb", name="outsb")
            nc.vector.tensor_scalar(outsb[:], out2[0:2, 0:PB], fc2b[:], None, OP.add)
            nc.sync.dma_start(d_out[:, :], outsb[:])

    nc.compile()
    return nc


def _host_prep(inputs):
    bf16 = _bf16()
    q1 = np.asarray(inputs["q1"])
    q2 = np.asarray(inputs["q2"])
    emb = np.asarray(inputs["emb"], np.float32)
    Ws = np.asarray(inputs["Ws"], np.float32)
    conv_k = np.asarray(inputs["conv_k"], np.float32)
    conv_b = np.asarray(inputs["conv_b"], np.float32)
    fc1_w = np.asarray(inputs["fc1_w"], np.float32)
    fc1_b = np.asarray(inputs["fc1_b"], np.float32)
    ln_g = np.asarray(inputs["ln_g"], np.float32)
    ln_b = np.asarray(inputs["ln_b"], np.float32)
    fc2_w = np.asarray(inputs["fc2_w"], np.float32)
    fc2_b = np.asarray(inputs["fc2_b"], np.float32)

    valid1 = (q1 != 0).astype(np.float32)     # [B, L]
    valid2 = (q2 != 0).astype(np.float32)
    e1 = emb[q1]                              # [B, L, D] fp32
    e2 = emb[q2]
    e1t = e1.transpose(0, 2, 1)               # [B, D, L]
    e2t = e2.transpose(0, 2, 1)

    # padded embeddings: [B, D, 2*WPAD], data at cols [2:258] per side
    es = np.zeros((B, D, 2 * WPAD), np.float32)
    es[:, :, 2:258] = e1t
    es[:, :, WPAD + 2:WPAD + 258] = e2t

    # v broadcast: [B, D, 512] = [v1 | v2] replicated over D
    vr = np.concatenate([valid1, valid2], axis=1)[:, None, :]        # [B,1,512]
    vrs = np.broadcast_to(vr, (B, D, 2 * L)).astype(bf16)

    # layer-0 sp: masked embeddings [B, D, 512]; sq0: -0.5*[sq1 | sq2] rows
    s1 = (e1t * valid1[:, None, :]).astype(bf16).astype(np.float32)
    s2 = (e2t * valid2[:, None, :]).astype(bf16).astype(np.float32)
    sq1 = (s1 * s1).sum(1)                    # [B, L]
    sq2 = (s2 * s2).sum(1)
    sp0 = np.concatenate([s1, s2], axis=2)    # [B, D, 512]
    sq0 = (-0.5 * np.concatenate([sq1, sq2], axis=1))[:, None, :]  # [B, 1, 512]

    # res pieces 0/3: initial embedding sums over L, [2, D, B] -> per core
    res0 = e1.sum(1).T.astype(np.float32)     # [D, B]
    res3 = e2.sum(1).T.astype(np.float32)

    # conv factorization: W chunks (lhsT for F), band matrices M (k1), E (k0)
    wch = np.zeros((NL, 2, 128, D), np.float32)
    mdy = np.zeros((NL, 3, D, D), np.float32)
    edy = np.zeros((NL, 3, D, D), np.float32)
    din = np.arange(D)
    for l in range(NL):
        wch[l, 0] = Ws[l][0:128]
        wch[l, 1] = Ws[l][128:256]
        k0 = conv_k[l, 0, 0]                  # e-channel [3,3]
        k1 = conv_k[l, 0, 1]                  # f-channel
        for idy in range(3):
            for dx in (-1, 0, 1):
                dout = din - dx
                m = (dout >= 0) & (dout < D)
                mdy[l, idy][din[m], dout[m]] = k1[idy, dx + 1]
                edy[l, idy][din[m], dout[m]] = k0[idy, dx + 1]

    fc1w = np.ascontiguousarray(fc1_w.reshape(6, D, LIN)).astype(bf16)
    fc2w = np.ascontiguousarray(fc2_w.reshape(4, 128, 2)).astype(bf16)

    common = {
        "wch": wch.astype(bf16),
        "mdy": mdy.astype(bf16),
        "edy": edy.astype(bf16),
        "fc1w": fc1w,
        "fc1b": np.tile(fc1_b[None, :], (PB, 1)).astype(bf16),
        "lng": np.tile(ln_g[None, :], (PB, 1)).astype(bf16),
        "lnb": np.tile(ln_b[None, :], (PB, 1)).astype(bf16),
        "fc2w": fc2w,
        "fc2b": np.ascontiguousarray(fc2_b.reshape(2, 1)).astype(np.float32),
        "ident32": np.eye(128, dtype=np.float32),
    }
    in_maps = []
    for c in range(NCORES):
        sl = slice(c * PB, (c + 1) * PB)
        m = dict(common)
        m["es"] = es[sl].astype(bf16)
        m["vrs"] = np.ascontiguousarray(vrs[sl])
        m["sp0"] = sp0[sl].astype(bf16)
        m["sq0"] = sq0[sl].astype(bf16)
        m["res03"] = np.stack([res0[:, sl], res3[:, sl]], 0)
        in_maps.append(m)
    return in_maps, [float(x) for x in conv_b.reshape(-1)[:NL]]


def _ensure_ntff_hook():
    """Register the axon NTFF profiling hook if antenv.axon_hooks is absent."""
    import types
    try:
        from antenv.axon_hooks import get_axon_ntff_profile_hook  # noqa
        return
    except ImportError:
        pass
    import antenv
    mod = types.ModuleType("antenv.axon_hooks")
    mod._hook = None
    def set_axon_ntff_profile_hook(h):
        mod._hook = h
    def get_axon_ntff_profile_hook():
        return mod._hook
    mod.set_axon_ntff_profile_hook = set_axon_ntff_profile_hook
    mod.get_axon_ntff_profile_hook = get_axon_ntff_profile_hook
    sys.modules["antenv.axon_hooks"] = mod
    antenv.axon_hooks = mod
    try:
        from trn_agent_boot.trn_boot import _ntff_profile_via_ctypes
        hook = _ntff_profile_via_ctypes("/opt/axon/libaxon_pjrt.so")
        if hook is not None:
            set_axon_ntff_profile_hook(hook)
    except Exception as e:
        print("ntff hook registration failed:", e)


def run(inputs, trace=False):
    from concourse import bass_utils
    if trace:
        _ensure_ntff_hook()
    in_maps, cb_vals = _host_prep(inputs)
    key = tuple(cb_vals)
    if key not in _CACHE:
        _CACHE[key] = _build(cb_vals)
    nc = _CACHE[key]
    res = bass_utils.run_bass_kernel_spmd(
        nc, in_maps, core_ids=list(range(NCORES)), trace=trace)
    out = np.concatenate([np.asarray(r["out"], np.float32).T
                          for r in res.results], 0)
    return out, res


def kernel(**inputs) -> np.ndarray:
    out, _ = run(inputs, trace=False)
    return out.astype(np.float32)


# revision 62
# speedup vs baseline: 1.0260x; 1.0260x over previous
"""ABCNN forward kernel for 8 Trainium2 NeuronCores (data-parallel over batch).

Hardcoded problem: B=256, L=256, D=100, V=50000, NUM_LAYER=2, LIN=512.
Each core processes 32 batch items end-to-end; no collectives.

Restructured vs the first version:
- d2 rank-1 terms folded into the cross matmul via 2 extra contraction rows
  (sp rows 100/101 hold [-0.5*sq1 | 1] and [1 | -0.5*sq2]); psum = -0.5*d2.
- A^T produced by swapped-operand matmuls + a second activation chain
  (no PE transposes of A).
- Conv factored as conv_f = sum_dy shift_dy(A @ W) @ M_dy: F = A@W computed
  once (2 matmuls, W stationary), then 3 banded matmuls + 3 e-channel banded
  matmuls, all N=512.
- Chain shortened: dmax = max(-2*psum, 0); r = ARS(dmax+eps); s = dmax*r;
  t = 2s + dmax; A = ARS(t + 1) = 1/(1+sqrt(dmax)).  Only ARS-set scalar
  functions; tanh grouped in per-16-batch waves to avoid table thrashing.
- Last layer skips match2/pooling/e-update entirely (dead code in reference).
- Host precomputes: layer-0 masked sp (incl. sq rows), v-broadcasts, initial
  e-means (res pieces 0/3), padded embeddings.
"""
import sys
import numpy as np

sys.path.insert(0, "/opt/trn_rl_repo")

B, L, D, V = 256, 256, 100, 50000
NL = 2
LIN = 512
EPS = 1e-5
NCORES = 8
PB = B // NCORES   # 32 batches per core
DEPS = 1e-12       # eps inside first abs_rsqrt of the chain
GRP = 16           # batches per scalar-table wave
WPAD = 260         # padded row width for shifted reads: [2 zeros, 256, 2 zeros]

_CACHE = {}


def _bf16():
    import ml_dtypes
    return ml_dtypes.bfloat16


def _build(cb_vals):
    """Build the Bass/Tile graph once. Returns nc."""
    import concourse.bass as bass
    import concourse.mybir as mybir
    from concourse import bacc, tile

    dt = mybir.dt
    AF = mybir.ActivationFunctionType
    OP = mybir.AluOpType
    AX = mybir.AxisListType
    ARS = AF.Abs_reciprocal_sqrt

    nc = bacc.Bacc("TRN2")

    # ---------------- DRAM inputs (per-core shard shapes) ------------------
    d_es = nc.dram_tensor("es", [PB, D, 2 * WPAD], dt.bfloat16, kind="ExternalInput")
    d_vrs = nc.dram_tensor("vrs", [PB, D, 2 * L], dt.bfloat16, kind="ExternalInput")
    d_sp0 = nc.dram_tensor("sp0", [PB, D, 2 * L], dt.bfloat16, kind="ExternalInput")
    d_sq0 = nc.dram_tensor("sq0", [PB, 1, 2 * L], dt.bfloat16, kind="ExternalInput")
    d_res03 = nc.dram_tensor("res03", [2, D, PB], dt.float32, kind="ExternalInput")
    d_wch = nc.dram_tensor("wch", [NL, 2, 128, D], dt.bfloat16, kind="ExternalInput")
    d_mdy = nc.dram_tensor("mdy", [NL, 3, D, D], dt.bfloat16, kind="ExternalInput")
    d_edy = nc.dram_tensor("edy", [NL, 3, D, D], dt.bfloat16, kind="ExternalInput")
    d_fc1w = nc.dram_tensor("fc1w", [6, D, LIN], dt.bfloat16, kind="ExternalInput")
    d_fc1b = nc.dram_tensor("fc1b", [PB, LIN], dt.bfloat16, kind="ExternalInput")
    d_lng = nc.dram_tensor("lng", [PB, LIN], dt.bfloat16, kind="ExternalInput")
    d_lnb = nc.dram_tensor("lnb", [PB, LIN], dt.bfloat16, kind="ExternalInput")
    d_fc2w = nc.dram_tensor("fc2w", [4, 128, 2], dt.bfloat16, kind="ExternalInput")
    d_fc2b = nc.dram_tensor("fc2b", [2, 1], dt.float32, kind="ExternalInput")
    d_ident32 = nc.dram_tensor("ident32", [128, 128], dt.float32, kind="ExternalInput")
    d_out = nc.dram_tensor("out", [2, PB], dt.float32, kind="ExternalOutput")

    with tile.TileContext(nc) as tc:
        with (
            tc.tile_pool(name="pers", bufs=1) as pers,
            tc.tile_pool(name="aa", bufs=GRP + 2) as aapool,
            tc.tile_pool(name="tr", bufs=2) as tr,
            tc.tile_pool(name="hd", bufs=1) as hd,
            tc.tile_pool(name="ps_a", bufs=4, space="PSUM") as ps_a,
            tc.tile_pool(name="ps_fc", bufs=2, space="PSUM") as ps_fc,
            tc.tile_pool(name="ps_sml", bufs=2, space="PSUM") as ps_sml,
        ):
            # ---------------- persistent tiles -----------------------------
            es, os_, vrs = [], [], []
            for b in range(PB):
                es.append(pers.tile([D, 2 * WPAD], dt.bfloat16, tag=f"e_{b}", name=f"e_{b}"))
                os_.append(pers.tile([D, 2 * L], dt.bfloat16, tag=f"o_{b}", name=f"o_{b}"))
                vrs.append(pers.tile([D, 2 * L], dt.bfloat16, tag=f"vr_{b}", name=f"vr_{b}"))
            wch = pers.tile([128, NL * 2 * D], dt.bfloat16, tag="wch", name="wch")
            mdy = pers.tile([D, NL * 3 * D], dt.bfloat16, tag="mdy", name="mdy")
            edy = pers.tile([D, NL * 3 * D], dt.bfloat16, tag="edy", name="edy")
            fc1w = pers.tile([D, 6 * LIN], dt.bfloat16, tag="fc1w", name="fc1w")
            fc1b = pers.tile([PB, LIN], dt.bfloat16, tag="fc1b", name="fc1b")
            lng = pers.tile([PB, LIN], dt.bfloat16, tag="lng", name="lng")
            lnb = pers.tile([PB, LIN], dt.bfloat16, tag="lnb", name="lnb")
            fc2w = pers.tile([128, 4 * 2], dt.bfloat16, tag="fc2w", name="fc2w")
            fc2b = pers.tile([2, 1], dt.float32, tag="fc2b", name="fc2b")
            ident32 = pers.tile([128, 128], dt.float32, tag="ident32", name="ident32")
            resX = [pers.tile([D, PB], dt.float32, tag=f"res_{p}", name=f"res_{p}")
                    for p in range(6)]
            # y and F staging buffers, double-buffered by batch parity,
            # with zeroed border columns for shifted reads.
            ybuf = [pers.tile([D, 2 * WPAD], dt.bfloat16, tag=f"y_{i}", name=f"y_{i}")
                    for i in range(3)]
            fsb = [pers.tile([D, 2 * WPAD], dt.bfloat16, tag=f"fs_{i}", name=f"fs_{i}")
                   for i in range(2)]
            ones_Dx1 = pers.tile([D, 1], dt.bfloat16, tag="c_onesd1", name="c_onesd1")
            ones_128x1 = pers.tile([128, 1], dt.bfloat16, tag="c_ones128", name="c_ones128")
            third_1xD = pers.tile([1, D], dt.bfloat16, tag="c_third", name="c_third")
            # ext-row operand pair for the rank-2 d2 term (K=33 matmul):
            # EL: row0 = -0.5*sq (per match), row32 = 1, rows 1..31 = 0
            # ER: row0 = 1, row32 = -0.5*sq (per match), rows 1..31 = 0
            ELs = [pers.tile([33, 2 * L], dt.bfloat16, tag=f"EL_{i}", name=f"EL_{i}")
                   for i in range(4)]
            ERs = [pers.tile([33, 2 * L], dt.bfloat16, tag=f"ER_{i}", name=f"ER_{i}")
                   for i in range(4)]
            c_deps = pers.tile([128, 1], dt.float32, tag="c_deps", name="c_deps")
            c_eps = pers.tile([128, 1], dt.float32, tag="c_eps", name="c_eps")
            c_cb = [pers.tile([128, 1], dt.float32, tag=f"c_cb{l}", name=f"c_cb{l}")
                    for l in range(NL)]

            nc.vector.memset(ones_Dx1[:], 1.0)
            nc.vector.memset(ones_128x1[:], 1.0)
            nc.vector.memset(third_1xD[:], 1.0 / 3.0)
            nc.vector.memset(c_deps[:], DEPS)
            nc.vector.memset(c_eps[:], EPS)
            for l in range(NL):
                nc.vector.memset(c_cb[l][:], float(cb_vals[l]))
            for i in range(3):
                nc.vector.memset(ybuf[i][:], 0.0)
            for i in range(2):
                nc.vector.memset(fsb[i][:], 0.0)
            for i in range(4):
                nc.vector.memset(ELs[i][:], 0.0)
                nc.vector.memset(ERs[i][:], 0.0)
                nc.vector.memset(ELs[i][32:33, :], 1.0)
                nc.vector.memset(ERs[i][0:1, :], 1.0)

            # ---------------- load weights (es/vrs staggered into waves) ---
            for l in range(NL):
                for c in range(2):
                    nc.sync.dma_start(
                        wch[:, (l * 2 + c) * D:(l * 2 + c + 1) * D],
                        d_wch[l, c, :, :])
                for dy in range(3):
                    nc.sync.dma_start(
                        mdy[:, (l * 3 + dy) * D:(l * 3 + dy + 1) * D],
                        d_mdy[l, dy, :, :])
                    nc.sync.dma_start(
                        edy[:, (l * 3 + dy) * D:(l * 3 + dy + 1) * D],
                        d_edy[l, dy, :, :])
            for p in range(6):
                nc.sync.dma_start(fc1w[:, p * LIN:(p + 1) * LIN], d_fc1w[p, :, :])
            nc.sync.dma_start(fc1b[:], d_fc1b[:, :])
            nc.sync.dma_start(lng[:], d_lng[:, :])
            nc.sync.dma_start(lnb[:], d_lnb[:, :])
            for q in range(4):
                nc.sync.dma_start(fc2w[:, q * 2:(q + 1) * 2], d_fc2w[q, :, :])
            nc.sync.dma_start(fc2b[:], d_fc2b[:, :])
            nc.sync.dma_start(ident32[:], d_ident32[:, :])
            nc.sync.dma_start(resX[0][:], d_res03[0, :, :])
            nc.sync.dma_start(resX[3][:], d_res03[1, :, :])

            # ---------------- helpers --------------------------------------
            def chain(dp, out_ap, mid_on_gpsimd, clamp_on_scalar=False):
                """A = 1/(1+sqrt(max(-2*psum,0))) from a [128,512] PSUM tile;
                writes bf16 A into out_ap."""
                eng = nc.gpsimd if mid_on_gpsimd else nc.vector
                dmax = tr.tile([128, 2 * L], dt.bfloat16, tag="dmax", name="dmax", bufs=3)
                if clamp_on_scalar:
                    nc.scalar.activation(dmax[:], dp[:], AF.Relu, scale=-2.0)
                else:
                    nc.vector.tensor_scalar(dmax[:], dp[:], -2.0, 0.0, OP.mult, OP.max)
                # r = 2/sqrt(dmax) via ARS(0.25*x); s = dmax*r = 2*sqrt(dmax)
                r = tr.tile([128, 2 * L], dt.bfloat16, tag="chr", name="chr", bufs=3)
                nc.scalar.activation(r[:], dmax[:], ARS, bias=c_deps[:], scale=0.25)
                s = tr.tile([128, 2 * L], dt.bfloat16, tag="chs", name="chs", bufs=3)
                eng.tensor_tensor(s[:], dmax[:], r[:], OP.mult)
                t = tr.tile([128, 2 * L], dt.bfloat16, tag="cht", name="cht", bufs=3)
                eng.tensor_tensor(t[:], s[:], dmax[:], OP.add)
                nc.scalar.activation(out_ap, t[:], ARS, bias=1.0)

            def ext_fill(b, spq_src):
                """sq-sums of the match operands -> EL row0 / ER row32."""
                el, er = ELs[(b % 2) + 2 * (b // GRP)], ERs[(b % 2) + 2 * (b // GRP)]
                sqt = ps_sml.tile([32, 2 * L], dt.float32, tag="sml", name="sqt")
                nc.tensor.matmul(sqt[0:1, :], ones_Dx1[:], spq_src[:],
                                 start=True, stop=True)
                nc.vector.tensor_scalar_mul(el[0:1, :], sqt[0:1, :], -0.5)
                nc.vector.tensor_scalar_mul(er[32:33, :], sqt[0:1, :], -0.5)
                return el, er

            def d2_mms(dp, sp, el, er, transposed):
                """psum = -0.5 * d2 via cross (K=100) + ext (K=33) matmuls."""
                src, dst = (L, 0) if transposed else (0, L)
                for c in range(2):
                    sl = dp[:, c * L:(c + 1) * L]
                    nc.tensor.matmul(sl, sp[:, src + c * 128:src + (c + 1) * 128],
                                     sp[:, dst:dst + L], start=True, stop=False)
                    nc.tensor.matmul(sl, el[:, src + c * 128:src + (c + 1) * 128],
                                     er[:, dst:dst + L], start=False, stop=True)

            # ---------------- per-batch stage emitters ----------------------
            aas = {}

            def p1_batch(l, b, clamp_on_scalar=False):
                """Match on embeddings -> A and A^T (ARS-set scalar only)."""
                sp = tr.tile([D, 2 * L], dt.bfloat16, tag="sp",
                             name="sp", bufs=6)
                if l == 0:
                    nc.sync.dma_start(sp[:], d_sp0[b, :, :])
                    el = ELs[(b % 2) + 2 * (b // GRP)]
                    er = ERs[(b % 2) + 2 * (b // GRP)]
                    nc.sync.dma_start(el[0:1, :], d_sq0[b, :, :])
                    nc.sync.dma_start(er[32:33, :], d_sq0[b, :, :])
                else:
                    ev = es[b][:].rearrange("p (c w) -> p c w", c=2)
                    vv = vrs[b][:].rearrange("p (c w) -> p c w", c=2)
                    spv = sp[:].rearrange("p (c w) -> p c w", c=2)
                    nc.vector.tensor_tensor(spv, ev[:, :, 2:258], vv, OP.mult)
                    spq = tr.tile([D, 2 * L], dt.bfloat16, tag="spq",
                                  name="spq", bufs=2)
                    nc.gpsimd.tensor_tensor(spq[:], sp[:], sp[:], OP.mult)
                    el, er = ext_fill(b, spq)
                aa = aapool.tile([128, 4 * L], dt.bfloat16, tag="AA", name="AA")
                aas[(l, b)] = aa
                # A: psum[i_chunk, j] ; A^T: psum[j_chunk, i]
                dp = ps_a.tile([128, 2 * L], dt.float32, tag="d2", name="dp")
                d2_mms(dp, sp, el, er, transposed=False)
                chain(dp, aa[:, 2 * L:4 * L], mid_on_gpsimd=False,
                      clamp_on_scalar=clamp_on_scalar)
                dpT = ps_a.tile([128, 2 * L], dt.float32, tag="d2", name="dpT")
                d2_mms(dpT, sp, el, er, transposed=True)
                chain(dpT, aa[:, 0:2 * L], mid_on_gpsimd=True,
                      clamp_on_scalar=clamp_on_scalar)

            def conv_batch(l, b):
                """F = A@W, banded conv, tanh (tanh-set scalar only)."""
                if True:
                    if True:
                        aa = aas[(l, b)]
                        fpt = ps_fc.tile([128, 2 * L], dt.float32, tag="fcv", name="fpt")
                        fp = fpt[0:D, :]
                        fpv = fp.rearrange("p (c w) -> p c w", c=2)
                        aav = aa[:].rearrange("p (x y w) -> p x y w", x=2, y=2)
                        for c in range(2):
                            # rhs: [128, 2, 256] = {AT chunk c, A chunk c}
                            nc.tensor.matmul(
                                fpv, wch[:, (l * 2 + c) * D:(l * 2 + c + 1) * D],
                                aav[:, :, c, :], start=(c == 0), stop=(c == 1))
                        fs = fsb[b % 2]
                        fsv = fs[:].rearrange("p (c w) -> p c w", c=2)
                        nc.vector.tensor_copy(fsv[:, :, 2:258], fpv)
                        cvt = ps_fc.tile([128, 2 * L], dt.float32, tag="fcv", name="cvt")
                        cv = cvt[0:D, :]
                        cvv = cv.rearrange("p (c w) -> p c w", c=2)
                        ev = es[b][:].rearrange("p (c w) -> p c w", c=2)
                        for dy in range(3):
                            nc.tensor.matmul(
                                cvv, mdy[:, (l * 3 + dy) * D:(l * 3 + dy + 1) * D],
                                fsv[:, :, 1 + dy:257 + dy],
                                start=(dy == 0), stop=False)
                            nc.tensor.matmul(
                                cvv, edy[:, (l * 3 + dy) * D:(l * 3 + dy + 1) * D],
                                ev[:, :, 1 + dy:257 + dy],
                                start=False, stop=(dy == 2))
                        for (side, pieceidx) in ((0, 1 + l), (1, 4 + l)):
                            nc.scalar.activation(
                                os_[b][:, side * L:(side + 1) * L],
                                cv[:, side * L:(side + 1) * L], AF.Tanh,
                                bias=c_cb[l][0:D],
                                accum_out=resX[pieceidx][:, b:b + 1])

            p3state = {}

            def p3_front(b):
                """Match on conv outputs through the A2 chain (ARS-set only)."""
                if True:
                    for _ in range(1):
                        sp = tr.tile([D, 2 * L], dt.bfloat16, tag="sp",
                                     name="sp", bufs=6)
                        nc.vector.tensor_tensor(sp[:], os_[b][:], vrs[b][:],
                                                OP.mult)
                        spq = tr.tile([D, 2 * L], dt.bfloat16, tag="spq",
                                      name="spq", bufs=2)
                        nc.gpsimd.tensor_tensor(spq[:], sp[:], sp[:], OP.mult)
                        el, er = ext_fill(b, spq)
                        dp2 = ps_a.tile([128, 2 * L], dt.float32, tag="d2", name="dp2")
                        d2_mms(dp2, sp, el, er, transposed=False)
                        # chain with mids on gpsimd; final split w/ accum -> w1c
                        dmax = tr.tile([128, 2 * L], dt.bfloat16, tag="dmax",
                                       name="dmax", bufs=3)
                        nc.scalar.activation(dmax[:], dp2[:], AF.Relu, scale=-2.0)
                        r = tr.tile([128, 2 * L], dt.bfloat16, tag="chr",
                                    name="chr", bufs=3)
                        nc.scalar.activation(r[:], dmax[:], ARS, bias=c_deps[:],
                                             scale=0.25)
                        s = tr.tile([128, 2 * L], dt.bfloat16, tag="chs3",
                                    name="chs3", bufs=2)
                        nc.vector.tensor_tensor(s[:], dmax[:], r[:], OP.mult)
                        t = tr.tile([128, 2 * L], dt.bfloat16, tag="cht3",
                                    name="cht3", bufs=2)
                        nc.gpsimd.tensor_tensor(t[:], s[:], dmax[:], OP.add)
                        a2 = tr.tile([128, 2 * L], dt.bfloat16, tag="A2",
                                     name="a2", bufs=4)
                        w1c = tr.tile([128, 2], dt.float32, tag="w1c",
                                      name="w1c", bufs=4)
                        for c in range(2):
                            nc.scalar.activation(
                                a2[:, c * L:(c + 1) * L], t[:, c * L:(c + 1) * L],
                                ARS, bias=1.0, accum_out=w1c[:, c:c + 1])
                        p3state[b] = (a2, w1c)

            def p3_tail(b):
                """w1/w2 weighting, pooling, e-update for an earlier p3_front."""
                a2, w1c = p3state.pop(b)
                if True:
                    for _ in range(1):
                        # w2 = col sums of A2 (over i), via ones matmuls
                        w2t = ps_sml.tile([32, 2 * L], dt.float32, tag="sml",
                                          name="w2t")
                        nc.tensor.matmul(w2t[0:1, 0:L], ones_128x1[:], a2[:, 0:L],
                                         start=True, stop=False)
                        nc.tensor.matmul(w2t[0:1, 0:L], ones_128x1[:], a2[:, L:2 * L],
                                         start=False, stop=True)
                        w2row = tr.tile([1, L], dt.bfloat16, tag="w2row", name="w2row")
                        nc.vector.tensor_copy(w2row[:], w2t[0:1, 0:L])
                        # w1 rows via two fp32 column transposes
                        w1row = tr.tile([1, L], dt.bfloat16, tag="w1row", name="w1row")
                        for c in range(2):
                            w1t = ps_sml.tile([32, 2 * L], dt.float32, tag="sml",
                                              name="w1t")
                            nc.tensor.transpose(w1t[0:1, 0:128], w1c[:, c:c + 1],
                                                ident32[:])
                            nc.vector.tensor_copy(w1row[0:1, c * 128:(c + 1) * 128],
                                                  w1t[0:1, 0:128])
                        # wr = (1/3) * [w1 bcast | w2 bcast] over partitions
                        wr = ps_sml.tile([D, 2 * L], dt.float32, tag="sml", name="wr")
                        nc.tensor.matmul(wr[:, 0:L], third_1xD[:], w1row[:],
                                         start=True, stop=True)
                        nc.tensor.matmul(wr[:, L:2 * L], third_1xD[:], w2row[:],
                                         start=True, stop=True)
                        # y = o * wr (into padded buffer), pool, e += pooled
                        y = ybuf[b % 3]
                        yv = y[:].rearrange("p (c w) -> p c w", c=2)
                        ov = os_[b][:].rearrange("p (c w) -> p c w", c=2)
                        wv = wr[:].rearrange("p (c w) -> p c w", c=2)
                        nc.vector.tensor_tensor(yv[:, :, 2:258], ov, wv, OP.mult)
                        u = tr.tile([D, 2 * L], dt.bfloat16, tag="u", name="u", bufs=3)
                        uv = u[:].rearrange("p (c w) -> p c w", c=2)
                        nc.gpsimd.tensor_tensor(uv, yv[:, :, 1:257], yv[:, :, 3:259],
                                                OP.add)
                        u2 = tr.tile([D, 2 * L], dt.bfloat16, tag="u2", name="u2",
                                     bufs=3)
                        u2v = u2[:].rearrange("p (c w) -> p c w", c=2)
                        nc.gpsimd.tensor_tensor(u2v, uv, yv[:, :, 2:258], OP.add)
                        evv = es[b][:].rearrange("p (c w) -> p c w", c=2)[:, :, 2:258]
                        nc.vector.tensor_tensor(evv, evv, u2v, OP.add)

            # ---------------- wave sequence --------------------------------
            # Scalar activations are grouped by table set per wave (ARS vs
            # tanh); no_sync_barrier keeps the scheduler from interleaving
            # them, which would thrash the activation-table RAM.  P3 of one
            # group shares a wave with the NEXT group's P1 (both ARS-only)
            # so the PE always has independent matmuls queued.
            g0 = list(range(GRP))
            g1 = list(range(GRP, PB))
            tc.no_sync_barrier()
            for b in g0:
                p1_batch(0, b)
            tc.no_sync_barrier()
            # stagger the non-critical input DMAs behind the first wave
            for b in g0:
                nc.sync.dma_start(es[b][:], d_es[b, :, :])
            for b in g0:
                conv_batch(0, b)
            tc.no_sync_barrier()
            for b in g1:
                nc.sync.dma_start(es[b][:], d_es[b, :, :])
            for b in range(PB):
                nc.sync.dma_start(vrs[b][:], d_vrs[b, :, :])
            # P3 tails run one pair late so their matmuls never block the
            # in-order PE queue on unresolved chain dependencies.
            for i, b in enumerate(g0):
                p1_batch(0, g1[i], clamp_on_scalar=True)
                p3_front(b)
                if i > 1:
                    p3_tail(g0[i - 2])
            p3_tail(g0[-2])
            p3_tail(g0[-1])
            tc.no_sync_barrier()
            for b in g1:
                conv_batch(0, b)
            tc.no_sync_barrier()
            for i, b in enumerate(g1):
                p1_batch(1, g0[i], clamp_on_scalar=True)
                p3_front(b)
                if i > 1:
                    p3_tail(g1[i - 2])
            p3_tail(g1[-2])
            p3_tail(g1[-1])
            tc.no_sync_barrier()
            for b in g0:
                conv_batch(1, b)
            tc.no_sync_barrier()
            for b in g1:
                p1_batch(1, b, clamp_on_scalar=True)
            tc.no_sync_barrier()
            for b in g1:
                conv_batch(1, b)
            tc.no_sync_barrier()

            # ---------------- output head ---------------------------------
            hps = ps_sml.tile([32, 2 * L], dt.float32, tag="sml", name="hps")
            for p in range(6):
                rb = hd.tile([D, PB], dt.bfloat16, tag="resbf", name="resbf")
                nc.vector.tensor_scalar_mul(rb[:], resX[p][:], 1.0 / 256.0)
                nc.tensor.matmul(hps[:], rb[:], fc1w[:, p * LIN:(p + 1) * LIN],
                                 start=(p == 0), stop=(p == 5))
            hb = hd.tile([PB, LIN], dt.float32, tag="lnA", name="hb")
            nc.vector.tensor_tensor(hb[:], hps[:], fc1b[:], OP.add)
            musum = hd.tile([PB, 1], dt.float32, tag="musum", name="musum")
            nc.vector.tensor_reduce(musum[:], hb[:], AX.X, OP.add)
            mu = hd.tile([PB, 1], dt.float32, tag="mu", name="mu")
            nc.vector.tensor_scalar_mul(mu[:], musum[:], 1.0 / LIN)
            hc = hd.tile([PB, LIN], dt.float32, tag="lnB", name="hc")
            nc.vector.tensor_scalar(hc[:], hb[:], mu[:], None, OP.subtract)
            sq = hd.tile([PB, LIN], dt.float32, tag="lnA", name="sqh")
            nc.vector.tensor_tensor(sq[:], hc[:], hc[:], OP.mult)
            vsum = hd.tile([PB, 1], dt.float32, tag="vsum", name="vsum")
            nc.vector.tensor_reduce(vsum[:], sq[:], AX.X, OP.add)
            rstd = hd.tile([PB, 1], dt.float32, tag="rstd", name="rstd")
            nc.scalar.activation(rstd[:], vsum[:], ARS, bias=c_eps[0:PB], scale=1.0 / LIN)
            hn = hd.tile([PB, LIN], dt.float32, tag="lnA", name="hn")
            nc.vector.tensor_scalar(hn[:], hc[:], rstd[:], None, OP.mult)
            hg = hd.tile([PB, LIN], dt.float32, tag="lnB", name="hg")
            nc.vector.tensor_tensor(hg[:], hn[:], lng[:], OP.mult)
            hgb = hd.tile([PB, LIN], dt.float32, tag="lnA", name="hgb")
            nc.vector.tensor_tensor(hgb[:], hg[:], lnb[:], OP.add)
            hr = hd.tile([PB, LIN], dt.float32, tag="hr", name="hr")
            nc.vector.tensor_scalar_max(hr[:], hgb[:], 0.0)
            out2 = ps_sml.tile([32, 2 * L], dt.float32, tag="sml", name="out2")
            for q in range(4):
                htp = ps_fc.tile([128, 2 * L], dt.float32, tag="fcv", name="htp")
                nc.tensor.transpose(htp[:, 0:PB], hr[:, q * 128:(q + 1) * 128],
                                    ident32[0:PB, 0:PB])
                ht = hd.tile([128, PB], dt.bfloat16, tag="ht", name="ht")
                nc.vector.tensor_copy(ht[:], htp[:, 0:PB])
                nc.tensor.matmul(out2[0:2, 0:PB], fc2w[:, q * 2:(q + 1) * 2], ht[:],
                                 start=(q == 0), stop=(q == 3))
            outsb = hd.tile([2, PB], dt.float32, tag="outsb", name="outsb")
            nc.vector.tensor_scalar(outsb[:], out2[0:2, 0:PB], fc2b[:], None, OP.add)
            nc.sync.dma_start(d_out[:, :], outsb[:])

    nc.compile()
    return nc


def _host_prep(inputs):
    bf16 = _bf16()
    q1 = np.asarray(inputs["q1"])
    q2 = np.asarray(inputs["q2"])
    emb = np.asarray(inputs["emb"], np.float32)
    Ws = np.asarray(inputs["Ws"], np.float32)
    conv_k = np.asarray(inputs["conv_k"], np.float32)
    conv_b = np.asarray(inputs["conv_b"], np.float32)
    fc1_w = np.asarray(inputs["fc1_w"], np.float32)
    fc1_b = np.asarray(inputs["fc1_b"], np.float32)
    ln_g = np.asarray(inputs["ln_g"], np.float32)
    ln_b = np.asarray(inputs["ln_b"], np.float32)
    fc2_w = np.asarray(inputs["fc2_w"], np.float32)
    fc2_b = np.asarray(inputs["fc2_b"], np.float32)

    valid1 = (q1 != 0).astype(np.float32)     # [B, L]
    valid2 = (q2 != 0).astype(np.float32)
    e1 = emb[q1]                              # [B, L, D] fp32
    e2 = emb[q2]
    e1t = e1.transpose(0, 2, 1)               # [B, D, L]
    e2t = e2.transpose(0, 2, 1)

    # padded embeddings: [B, D, 2*WPAD], data at cols [2:258] per side
    es = np.zeros((B, D, 2 * WPAD), np.float32)
    es[:, :, 2:258] = e1t
    es[:, :, WPAD + 2:WPAD + 258] = e2t

    # v broadcast: [B, D, 512] = [v1 | v2] replicated over D
    vr = np.concatenate([valid1, valid2], axis=1)[:, None, :]        # [B,1,512]
    vrs = np.broadcast_to(vr, (B, D, 2 * L)).astype(bf16)

    # layer-0 sp: masked embeddings [B, D, 512]; sq0: -0.5*[sq1 | sq2] rows
    s1 = (e1t * valid1[:, None, :]).astype(bf16).astype(np.float32)
    s2 = (e2t * valid2[:, None, :]).astype(bf16).astype(np.float32)
    sq1 = (s1 * s1).sum(1)                    # [B, L]
    sq2 = (s2 * s2).sum(1)
    sp0 = np.concatenate([s1, s2], axis=2)    # [B, D, 512]
    sq0 = (-0.5 * np.concatenate([sq1, sq2], axis=1))[:, None, :]  # [B, 1, 512]

    # res pieces 0/3: initial embedding sums over L, [2, D, B] -> per core
    res0 = e1.sum(1).T.astype(np.float32)     # [D, B]
    res3 = e2.sum(1).T.astype(np.float32)

    # conv factorization: W chunks (lhsT for F), band matrices M (k1), E (k0)
    wch = np.zeros((NL, 2, 128, D), np.float32)
    mdy = np.zeros((NL, 3, D, D), np.float32)
    edy = np.zeros((NL, 3, D, D), np.float32)
    din = np.arange(D)
    for l in range(NL):
        wch[l, 0] = Ws[l][0:128]
        wch[l, 1] = Ws[l][128:256]
        k0 = conv_k[l, 0, 0]                  # e-channel [3,3]
        k1 = conv_k[l, 0, 1]                  # f-channel
        for idy in range(3):
            for dx in (-1, 0, 1):
                dout = din - dx
                m = (dout >= 0) & (dout < D)
                mdy[l, idy][din[m], dout[m]] = k1[idy, dx + 1]
                edy[l, idy][din[m], dout[m]] = k0[idy, dx + 1]

    fc1w = np.ascontiguousarray(fc1_w.reshape(6, D, LIN)).astype(bf16)
    fc2w = np.ascontiguousarray(fc2_w.reshape(4, 128, 2)).astype(bf16)

    common = {
        "wch": wch.astype(bf16),
        "mdy": mdy.astype(bf16),
        "edy": edy.astype(bf16),
        "fc1w": fc1w,
        "fc1b": np.tile(fc1_b[None, :], (PB, 1)).astype(bf16),
        "lng": np.tile(ln_g[None, :], (PB, 1)).astype(bf16),
        "lnb": np.tile(ln_b[None, :], (PB, 1)).astype(bf16),
        "fc2w": fc2w,
        "fc2b": np.ascontiguousarray(fc2_b.reshape(2, 1)).astype(np.float32),
        "ident32": np.eye(128, dtype=np.float32),
    }
    in_maps = []
    for c in range(NCORES):
        sl = slice(c * PB, (c + 1) * PB)
        m = dict(common)
        m["es"] = es[sl].astype(bf16)
        m["vrs"] = np.ascontiguousarray(vrs[sl])
        m["sp0"] = sp0[sl].astype(bf16)
        m["sq0"] = sq0[sl].astype(bf16)
        m["res03"] = np.stack([res0[:, sl], res3[:, sl]], 0)
        in_maps.append(m)
    return in_maps, [float(x) for x in conv_b.reshape(-1)[:NL]]


def _ensure_ntff_hook():
    """Register the axon NTFF profiling hook if antenv.axon_hooks is absent."""
    import types
    try:
        from antenv.axon_hooks import get_axon_ntff_profile_hook  # noqa
        return
    except ImportError:
        pass
    import antenv
    mod = types.ModuleType("antenv.axon_hooks")
    mod._hook = None
    def set_axon_ntff_profile_hook(h):
        mod._hook = h
    def get_axon_ntff_profile_hook():
        return mod._hook
    mod.set_axon_ntff_profile_hook = set_axon_ntff_profile_hook
    mod.get_axon_ntff_profile_hook = get_axon_ntff_profile_hook
    sys.modules["antenv.axon_hooks"] = mod
    antenv.axon_hooks = mod
    try:
        from trn_agent_boot.trn_boot import _ntff_profile_via_ctypes
        hook = _ntff_profile_via_ctypes("/opt/axon/libaxon_pjrt.so")
        if hook is not None:
            set_axon_ntff_profile_hook(hook)
    except Exception as e:
        print("ntff hook registration failed:", e)


def run(inputs, trace=False):
    from concourse import bass_utils
    if trace:
        _ensure_ntff_hook()
    in_maps, cb_vals = _host_prep(inputs)
    key = tuple(cb_vals)
    if key not in _CACHE:
        _CACHE[key] = _build(cb_vals)
    nc = _CACHE[key]
    res = bass_utils.run_bass_kernel_spmd(
        nc, in_maps, core_ids=list(range(NCORES)), trace=trace)
    out = np.concatenate([np.asarray(r["out"], np.float32).T
                          for r in res.results], 0)
    return out, res


def kernel(**inputs) -> np.ndarray:
    out, _ = run(inputs, trace=False)
    return out.astype(np.float32)
